# revision 41
# baseline (speedup 1.0000x reference)
"""Trainium2 Bass kernel for nn_ComplexEMA.

Math: reference computes, per (batch b, channel d), a causal convolution of
x[b,d,:] with a kernel k_d built from N=16 decaying complex exponentials
(radius <= ~0.86 for this model's parameters), plus a residual omega_d*x.
radius^128 < 1e-8, so k_d is representable by its first 128 taps.

Algorithm: chunk L=2048 into 16 chunks of C=128.  Then
    y[b,d,c*C+t] = sum_t' x[b,d,c*C+t']   * M0_d[t',t]
                 + sum_t' x[b,d,(c-1)C+t']* M1_d[t',t]
with M0_d[t',t] = k_d[t-t'] for t>=t' (+ omega_d on the diagonal) and
M1_d[t',t] = k_d[C+t-t'], significant only for t<t' (taps 1..127).

HBM cut: both matrices derive from the circulant P[t',t] = k[(t-t') mod C],
whose column halves are partition-rotations: P[p, t+64] = P[(p-64)%128, t].
Only P's first 64 columns (the seed, 2.10 MB/core) ship from HBM instead
of dense P (4.19 MB/core); the per-core DMA subsystem (~360 GB/s, charged
for SBUF-SBUF copies exactly like HBM traffic - measured) is the
bottleneck, so the rest is built by compute engines.  Block structure of
the halves (h0 = cols t<64, h1 = t>=64), with pg = P.h0:
    m1.h0 = strict-lower(pg)            (GpSimd affine_select, iota p-q>0)
    m0.h0 = pg - m1.h0                  (DVE subtract)
    m1.h1[64:128] = rot64(m1.h0[0:64]); m1.h1[0:64] = 0   (memset)
    m0.h1[0:64]   = rot64(pg[64:128])   (pure P values, mask all-true)
    m0.h1[64:128] = rot64(m0.h0[0:64])
P's second half is never materialized.  rot64 is the only cheap DVE
stream_shuffle (SBUF compute APs may start only at partition 0/32/64/96;
64->0 spans are legal).  m1/m0 live as four blocks of ONE tile
[m1h0|m0h0|m1h1|m0h1] so the two upper-row rotations batch into a single
shuffle; shuffle APs are bitcast to uint32 to halve element counts.
Per-group engine cost: DVE ~2.7us (2 shuffles + sub), GpSimd ~2.2us
(select + memset), both near the input-stream cadence; the DVE stream is
ordered so group g's ops depend only on seeds/gpsimd work that completed
a group earlier (no cross-engine ping-pong stalls).

Each channel is two 128x128 fp16 matmuls with the x-chunks as the
stationary operand ([t', (c,b)] layout) so PSUM lands in [(c,b), t] layout.
The matmul rhs reads M0/M1 through a [2,64]-strided block AP, keeping the
moving-column order t ascending.  8 channels accumulate into one 2-bank
PSUM tile (start=True per bank boundary); evacuation is one [128 x 1024]
scalar-engine copy per half-group, giving fine PE/ACT overlap and a short
last-group tail.  Seeds interleave with x per group on the input ring so
the first matmul starts ~14us in.

Sharding: channels D=1024 split across the 8 cores (128 each); batch stays
whole so every matmul has 128 moving columns.

Per-core DMA traffic: x 4.46 + seed 2.10 in, y 4.19 out = 10.75 MB
~= 30 us of DMA + ~8.6 us fixed init + tail.
"""

import math

import numpy as np

B, D, L, N = 8, 1024, 2048, 16
NCORES = 8
DLOC = D // NCORES          # 128 channels per core
C = 128                     # chunk size == significant taps
NCH = L // C                # 16 chunks
G = 16                      # channels per pipelined group
NGROUPS = DLOC // G
PADB = 8                    # zero columns ahead of each channel block
BLK = C + PADB              # 136 columns per channel block in SBUF
SW = 64                     # seed strip width (columns of P shipped)
HW_ = G * SW                # half width in a group tile's columns

_NC_CACHE = {}
LAST_EXEC_NS = None
LAST_RESULTS = None


def _host_weights(alpha, delta, theta, gamma_real, gamma_imag, omega):
    """Exact (float64) first 128 taps per channel, packed as the seed strip
    S[d, p, q] = P_d[p, q] = k[d, (q-p) mod C] for q<64, omega on tap 0."""
    sig = lambda v: 1.0 / (1.0 + np.exp(-v.astype(np.float64)))
    th = sig(theta[:, 0, 0]) * (2.0 * math.pi / N)            # (D,)
    wav = np.arange(1, N + 1, dtype=np.float64)
    phi = wav[None, :] * th[:, None]                          # (D,N)
    a = sig(alpha[:, :, 0])
    d_ = sig(delta[:, :, 0])
    radius = np.minimum(1.0 - a * d_, 1.0)
    gp = (gamma_real.astype(np.float64) + 1j * gamma_imag.astype(np.float64))
    gp *= math.sqrt(1.0 / N) * a
    q = radius * np.exp(1j * phi)                             # (D,N)

    taps = np.arange(C, dtype=np.float64)
    ql = q[:, :, None] ** taps[None, None, :]                 # (D,N,C)
    k = np.real((gp[:, :, None] * ql).sum(1))                 # (D,C)
    k[:, 0] += omega.astype(np.float64)

    t = np.arange(SW)
    p = np.arange(C)
    idx = (t[None, :] - p[:, None]) % C                       # (p,q)
    S = k[:, idx]                                             # (D,C,SW)
    return np.ascontiguousarray(S.astype(np.float16))


def _group_major(arr_core, inner):
    """[DLOC, C, inner] -> [NGROUPS, C, G*inner] contiguous."""
    return np.ascontiguousarray(
        arr_core.reshape(NGROUPS, G, C, inner)
        .transpose(0, 2, 1, 3)
        .reshape(NGROUPS, C, G * inner)
    )


def _build_nc():
    import concourse.bass as bass  # noqa: F401
    import concourse.mybir as mybir
    import concourse.tile as tile
    from concourse import bacc

    f16 = mybir.dt.float16
    u32 = mybir.dt.uint32
    f32 = mybir.dt.float32
    IDENT = list(range(32))

    nc = bacc.Bacc(None, target_bir_lowering=False)
    xt = nc.declare_dram_parameter("xt", [NGROUPS, C, G * BLK], f16, isOutput=False)
    sd = nc.declare_dram_parameter("seed", [NGROUPS, C, HW_], f16, isOutput=False)
    y = nc.declare_dram_parameter("y", [NGROUPS, NCH * B, G, C], f16, isOutput=True)

    def mviews(ap):
        # M cols = h*(2*HW_) + e*HW_ + g*SW + q, blocks [m1h0|m0h0|m1h1|m0h1]
        # -> [p, e, g, h, q]; rhs for (e, ch) = [p, 2, 64] in t order
        v = ap.rearrange("p (h e g q) -> p e g h q", h=2, e=2, q=SW)
        return v[:, 1], v[:, 0]          # m0 view, m1 view  [p, g, 2, SW]

    with tile.TileContext(nc) as tc:
        with (
            tc.tile_pool(name="xp", bufs=NGROUPS) as xp,
            tc.tile_pool(name="pp", bufs=NGROUPS) as pp,
            tc.tile_pool(name="mp", bufs=NGROUPS) as mp,
            tc.tile_pool(name="yp", bufs=3) as yp,
            tc.tile_pool(name="ps", bufs=4, space="PSUM") as ps,
        ):
            pgs, ms, xgs = [], [], []

            def issue_seed(g):
                pg = pp.tile([C, HW_], f16)
                nc.sync.dma_start(out=pg[:], in_=sd[g])
                pgs.append(pg)



            def issue_x(g):
                xg = xp.tile([C, G * BLK], f16)
                nc.sync.dma_start(out=xg[:], in_=xt[g])
                xgs.append(xg)

            def issue_seed2(g):
                # m0.h1[0:64] = rot64(P.h0[64:128]) = seed rows [64:128]:
                # re-read 128 KB from HBM on the scalar ring (idle until the
                # first evacuation) instead of a DVE stream_shuffle -- the
                # DVE prep chain is the critical path.  Allocates the M tile
                # so the later gpsimd select orders after this DMA, never
                # the other way round.
                mg = mp.tile([C, 4 * HW_], f16)
                ms.append(mg)
                nc.scalar.dma_start(
                    out=mg[0:64, 3 * HW_:4 * HW_], in_=sd[g, 64:128]
                )

            def prep_gpsimd(g):
                pg, mg = pgs[g], ms[g]
                # m1.h0 = strict-lower(P.h0): keep where p - q > 0
                nc.gpsimd.affine_select(
                    mg[:, 0:HW_],
                    pg[:],
                    pattern=[[0, G], [-1, SW]],
                    compare_op=mybir.AluOpType.is_gt,
                    fill=0.0,
                    base=0,
                    channel_multiplier=1,
                )
                # m1.h1 rows p<64 are identically zero
                nc.gpsimd.memset(mg[0:64, 2 * HW_:3 * HW_], 0)
                return

            def prep_sub(g):
                # m0.h0 = P.h0 - m1.h0 (DVE)
                mg = ms[g]
                nc.vector.tensor_tensor(
                    mg[:, HW_:2 * HW_], pgs[g][:], mg[:, 0:HW_],
                    mybir.AluOpType.subtract,
                )

            def prep_dve_shuf5(g):
                # m0.h1[0:64] = rot64(P.h0[64:128])  (pure P values)
                nc.vector.stream_shuffle(
                    ms[g][0:64, 3 * HW_:4 * HW_].bitcast(u32),
                    pgs[g][64:128, 0:HW_].bitcast(u32),
                    IDENT,
                )

            def prep_dve_batch(g):
                # [m1.h1|m0.h1][64:128] = rot64([m1.h0|m0.h0][0:64]) batched
                mg = ms[g]
                nc.vector.stream_shuffle(
                    mg[64:128, 2 * HW_:4 * HW_].bitcast(u32),
                    mg[0:64, 0:2 * HW_].bitcast(u32),
                    IDENT,
                )

            for g in range(NGROUPS):
                issue_seed(g)
                issue_seed2(g)
                prep_gpsimd(g)
                issue_x(g)
            # DVE stream: group 0's full prep first (earliest matmul start),
            # then the remaining pg-only rotations (never stall), then each
            # group's sub + batch rotation as its gpsimd select lands
            for g in range(NGROUPS):
                prep_sub(g)
                prep_dve_batch(g)
            for g in range(NGROUPS):
                xg, mg = xgs[g], ms[g]
                m0v, m1v = mviews(mg[:])
                ysb = yp.tile([C, G * C], f16)
                for half in range(2):
                    # two PSUM banks per tile; one [128 x 1024] evacuation
                    # per 8 channels balances ACT op count with finer
                    # PE/ACT overlap and a shorter last-group tail
                    yps = ps.tile([C, 8 * C], f32, tag="yps")
                    for j in range(8):
                        ch = half * 8 + j
                        base = ch * BLK
                        osl = slice(j * C, (j + 1) * C)
                        nc.tensor.matmul(
                            yps[:, osl],
                            lhsT=xg[:, base + PADB : base + PADB + C],
                            rhs=m0v[:, ch],
                            start=(j % 4 == 0),
                            stop=False,
                        )
                        # carry matmul: lhsT view shifted back one chunk (the
                        # leading 8 pad columns supply c=0's zero history)
                        nc.tensor.matmul(
                            yps[:, osl],
                            lhsT=xg[:, base : base + C],
                            rhs=m1v[:, ch],
                            start=False,
                            stop=(j % 4 == 3),
                        )
                    ysl = slice(half * 8 * C, (half + 1) * 8 * C)
                    nc.scalar.copy(ysb[:, ysl], yps[:])
                if g < NGROUPS - 1:
                    nc.scalar.dma_start(
                        out=y[g],
                        in_=ysb[:].rearrange("p (ch u) -> p ch u", u=C),
                    )
                else:
                    # split the final output DMA so the last HBM write (and
                    # its completion wait) is half-sized
                    half = G * C // 2
                    nc.scalar.dma_start(
                        out=y[g, :, : G // 2],
                        in_=ysb[:, :half].rearrange("p (ch u) -> p ch u", u=C),
                    )
                    nc.scalar.dma_start(
                        out=y[g, :, G // 2 :],
                        in_=ysb[:, half:].rearrange("p (ch u) -> p ch u", u=C),
                    )
    nc.compile()
    return nc


def _get_nc():
    if "nc" not in _NC_CACHE:
        _NC_CACHE["nc"] = _build_nc()
    return _NC_CACHE["nc"]


def kernel(x, alpha, delta, theta, gamma_real, gamma_imag, omega, **_):
    global LAST_EXEC_NS, LAST_RESULTS
    import os

    from concourse.bass_utils import run_bass_kernel_spmd

    x = np.asarray(x)
    sfull = _host_weights(
        np.asarray(alpha), np.asarray(delta), np.asarray(theta),
        np.asarray(gamma_real), np.asarray(gamma_imag), np.asarray(omega),
    )
    # x[b,d,c*C+t'] -> xt[d, t', pad8 + (c,b)] so each channel block's first
    # 8 columns are the zeros the carry matmul view needs.
    xtf = np.zeros((D, C, BLK), dtype=np.float16)
    xtf[:, :, PADB:] = (
        x.reshape(B, D, NCH, C).transpose(1, 3, 2, 0).reshape(D, C, NCH * B)
    )

    nc = _get_nc()
    in_maps = []
    for i in range(NCORES):
        sl = slice(i * DLOC, (i + 1) * DLOC)
        in_maps.append({
            "xt": _group_major(xtf[sl], BLK),
            "seed": _group_major(sfull[sl], SW),
        })
    trace = bool(int(os.environ.get("KERNEL_TRACE", "0")))
    res = run_bass_kernel_spmd(nc, in_maps, list(range(NCORES)), trace=trace)
    LAST_EXEC_NS = res.exec_time_ns
    LAST_RESULTS = res

    y = np.empty((B, D, L), dtype=np.float32)
    for i in range(NCORES):
        yi = res.results[i]["y"]                 # [NGROUPS, (c,b), G, C] fp16
        yi = yi.reshape(NGROUPS, NCH, B, G, C).transpose(2, 0, 3, 1, 4)
        y[:, i * DLOC : (i + 1) * DLOC, :] = (
            yi.reshape(B, DLOC, L).astype(np.float32)
        )
    return y


# revision 42
# speedup vs baseline: 1.1562x; 1.1562x over previous
"""Trainium2 Bass kernel for nn_ComplexEMA.

Math: reference computes, per (batch b, channel d), a causal convolution of
x[b,d,:] with a kernel k_d built from N=16 decaying complex exponentials
(radius <= ~0.86 for this model's parameters), plus a residual omega_d*x.
radius^128 < 1e-8, so k_d is representable by its first 128 taps.

Algorithm: chunk L=2048 into 16 chunks of C=128.  Then
    y[b,d,c*C+t] = sum_t' x[b,d,c*C+t']   * M0_d[t',t]
                 + sum_t' x[b,d,(c-1)C+t']* M1_d[t',t]
with M0_d[t',t] = k_d[t-t'] for t>=t' (+ omega_d on the diagonal) and
M1_d[t',t] = k_d[C+t-t'], significant only for t<t' (taps 1..127).

HBM cut: both matrices derive from the circulant P[t',t] = k[(t-t') mod C],
whose column halves are partition-rotations: P[p, t+64] = P[(p-64)%128, t].
Only P's first 64 columns (the seed, 2.10 MB/core) ship from HBM instead
of dense P (4.19 MB/core); the per-core DMA subsystem (~360 GB/s, charged
for SBUF-SBUF copies exactly like HBM traffic - measured) is the
bottleneck, so the rest is built by compute engines.  Block structure of
the halves (h0 = cols t<64, h1 = t>=64), with pg = P.h0:
    m1.h0 = strict-lower(pg)            (GpSimd affine_select, iota p-q>0)
    m0.h0 = pg - m1.h0                  (DVE subtract)
    m1.h1[64:128] = rot64(m1.h0[0:64]); m1.h1[0:64] = 0   (memset)
    m0.h1[0:64]   = rot64(pg[64:128])   (pure P values, mask all-true)
    m0.h1[64:128] = rot64(m0.h0[0:64])
P's second half is never materialized.  rot64 is the only cheap DVE
stream_shuffle (SBUF compute APs may start only at partition 0/32/64/96;
64->0 spans are legal).  m1/m0 live as four blocks of ONE tile
[m1h0|m0h0|m1h1|m0h1] so the two upper-row rotations batch into a single
shuffle; shuffle APs are bitcast to uint32 to halve element counts.
Per-group engine cost: DVE ~2.7us (2 shuffles + sub), GpSimd ~2.2us
(select + memset), both near the input-stream cadence; the DVE stream is
ordered so group g's ops depend only on seeds/gpsimd work that completed
a group earlier (no cross-engine ping-pong stalls).

Each channel is two 128x128 fp16 matmuls with the x-chunks as the
stationary operand ([t', (c,b)] layout) so PSUM lands in [(c,b), t] layout.
The matmul rhs reads M0/M1 through a [2,64]-strided block AP, keeping the
moving-column order t ascending.  8 channels accumulate into one 2-bank
PSUM tile (start=True per bank boundary); evacuation is one [128 x 1024]
scalar-engine copy per half-group, giving fine PE/ACT overlap and a short
last-group tail.  Seeds interleave with x per group on the input ring so
the first matmul starts ~14us in.

Sharding: channels D=1024 split across the 8 cores (128 each); batch stays
whole so every matmul has 128 moving columns.

Per-core DMA traffic: x 4.46 + seed 2.10 in, y 4.19 out = 10.75 MB
~= 30 us of DMA + ~8.6 us fixed init + tail.
"""

import math

import numpy as np

B, D, L, N = 8, 1024, 2048, 16
NCORES = 8
DLOC = D // NCORES          # 128 channels per core
C = 128                     # chunk size == significant taps
NCH = L // C                # 16 chunks
G = 16                      # channels per pipelined group
NGROUPS = DLOC // G
PADB = 8                    # zero columns ahead of each channel block
BLK = C + PADB              # 136 columns per channel block in SBUF
SW = 64                     # seed strip width (columns of P shipped)
HW_ = G * SW                # half width in a group tile's columns

_NC_CACHE = {}
LAST_EXEC_NS = None
LAST_RESULTS = None


def _host_weights(alpha, delta, theta, gamma_real, gamma_imag, omega):
    """Exact (float64) first 128 taps per channel, packed as the seed strip
    S[d, p, q] = P_d[p, q] = k[d, (q-p) mod C] for q<64, omega on tap 0."""
    sig = lambda v: 1.0 / (1.0 + np.exp(-v.astype(np.float64)))
    th = sig(theta[:, 0, 0]) * (2.0 * math.pi / N)            # (D,)
    wav = np.arange(1, N + 1, dtype=np.float64)
    phi = wav[None, :] * th[:, None]                          # (D,N)
    a = sig(alpha[:, :, 0])
    d_ = sig(delta[:, :, 0])
    radius = np.minimum(1.0 - a * d_, 1.0)
    gp = (gamma_real.astype(np.float64) + 1j * gamma_imag.astype(np.float64))
    gp *= math.sqrt(1.0 / N) * a
    q = radius * np.exp(1j * phi)                             # (D,N)

    taps = np.arange(C, dtype=np.float64)
    ql = q[:, :, None] ** taps[None, None, :]                 # (D,N,C)
    k = np.real((gp[:, :, None] * ql).sum(1))                 # (D,C)
    k[:, 0] += omega.astype(np.float64)

    t = np.arange(SW)
    p = np.arange(C)
    idx = (t[None, :] - p[:, None]) % C                       # (p,q)
    S = k[:, idx]                                             # (D,C,SW)
    return np.ascontiguousarray(S.astype(np.float16))


def _group_major(arr_core, inner):
    """[DLOC, C, inner] -> [NGROUPS, C, G*inner] contiguous."""
    return np.ascontiguousarray(
        arr_core.reshape(NGROUPS, G, C, inner)
        .transpose(0, 2, 1, 3)
        .reshape(NGROUPS, C, G * inner)
    )


def _build_nc():
    import concourse.bass as bass  # noqa: F401
    import concourse.mybir as mybir
    import concourse.tile as tile
    from concourse import bacc

    f16 = mybir.dt.float16
    u32 = mybir.dt.uint32
    f32 = mybir.dt.float32
    IDENT = list(range(32))

    nc = bacc.Bacc(None, target_bir_lowering=False)
    xt = nc.declare_dram_parameter("xt", [NGROUPS, C, G * BLK], f16, isOutput=False)
    sd = nc.declare_dram_parameter("seed", [NGROUPS, C, HW_], f16, isOutput=False)
    y = nc.declare_dram_parameter("y", [NGROUPS, NCH * B, G, C], f16, isOutput=True)

    def mviews(ap):
        # M cols = h*(2*HW_) + e*HW_ + g*SW + q, blocks [m1h0|m0h0|m1h1|m0h1]
        # -> [p, e, g, h, q]; rhs for (e, ch) = [p, 2, 64] in t order
        v = ap.rearrange("p (h e g q) -> p e g h q", h=2, e=2, q=SW)
        return v[:, 1], v[:, 0]          # m0 view, m1 view  [p, g, 2, SW]

    with tile.TileContext(nc) as tc:
        with (
            tc.tile_pool(name="xp", bufs=NGROUPS) as xp,
            tc.tile_pool(name="pp", bufs=NGROUPS) as pp,
            tc.tile_pool(name="mp", bufs=NGROUPS) as mp,
            tc.tile_pool(name="yp", bufs=3) as yp,
            tc.tile_pool(name="ps", bufs=4, space="PSUM") as ps,
        ):
            pgs, ms, xgs = [], [], []

            def issue_seed(g):
                pg = pp.tile([C, HW_], f16)
                nc.sync.dma_start(out=pg[:], in_=sd[g])
                pgs.append(pg)



            def issue_x(g):
                xg = xp.tile([C, G * BLK], f16)
                nc.sync.dma_start(out=xg[:], in_=xt[g])
                xgs.append(xg)

            def prep_gpsimd(g):
                pg = pgs[g]
                mg = mp.tile([C, 4 * HW_], f16)
                ms.append(mg)
                # m1.h0 = strict-lower(P.h0): keep where p - q > 0
                nc.gpsimd.affine_select(
                    mg[:, 0:HW_],
                    pg[:],
                    pattern=[[0, G], [-1, SW]],
                    compare_op=mybir.AluOpType.is_gt,
                    fill=0.0,
                    base=0,
                    channel_multiplier=1,
                )
                # m1.h1 rows p<64 are identically zero
                nc.gpsimd.memset(mg[0:64, 2 * HW_:3 * HW_], 0)

            def prep_sub(g):
                # m0.h0 = P.h0 - m1.h0 (DVE)
                mg = ms[g]
                nc.vector.tensor_tensor(
                    mg[:, HW_:2 * HW_], pgs[g][:], mg[:, 0:HW_],
                    mybir.AluOpType.subtract,
                )

            def prep_dve_shuf5(g):
                # m0.h1[0:64] = rot64(P.h0[64:128])  (pure P values)
                nc.vector.stream_shuffle(
                    ms[g][0:64, 3 * HW_:4 * HW_].bitcast(u32),
                    pgs[g][64:128, 0:HW_].bitcast(u32),
                    IDENT,
                )

            def prep_dve_batch(g):
                # [m1.h1|m0.h1][64:128] = rot64([m1.h0|m0.h0][0:64]) batched
                mg = ms[g]
                nc.vector.stream_shuffle(
                    mg[64:128, 2 * HW_:4 * HW_].bitcast(u32),
                    mg[0:64, 0:2 * HW_].bitcast(u32),
                    IDENT,
                )

            for g in range(NGROUPS):
                issue_seed(g)
                prep_gpsimd(g)
                issue_x(g)
            # DVE stream: group 0's full prep first (earliest matmul start),
            # then the remaining pg-only rotations (never stall), then each
            # group's sub + batch rotation as its gpsimd select lands
            prep_dve_shuf5(0)
            prep_sub(0)
            prep_dve_batch(0)
            for g in range(1, NGROUPS):
                prep_dve_shuf5(g)
            for g in range(1, NGROUPS):
                prep_sub(g)
                prep_dve_batch(g)
            for g in range(NGROUPS):
                xg, mg = xgs[g], ms[g]
                m0v, m1v = mviews(mg[:])
                ysb = yp.tile([C, G * C], f16)
                for half in range(2):
                    # two PSUM banks per tile; one [128 x 1024] evacuation
                    # per 8 channels balances ACT op count with finer
                    # PE/ACT overlap and a shorter last-group tail
                    yps = ps.tile([C, 8 * C], f32, tag="yps")
                    for j in range(8):
                        ch = half * 8 + j
                        base = ch * BLK
                        osl = slice(j * C, (j + 1) * C)
                        nc.tensor.matmul(
                            yps[:, osl],
                            lhsT=xg[:, base + PADB : base + PADB + C],
                            rhs=m0v[:, ch],
                            start=(j % 4 == 0),
                            stop=False,
                        )
                        # carry matmul: lhsT view shifted back one chunk (the
                        # leading 8 pad columns supply c=0's zero history)
                        nc.tensor.matmul(
                            yps[:, osl],
                            lhsT=xg[:, base : base + C],
                            rhs=m1v[:, ch],
                            start=False,
                            stop=(j % 4 == 3),
                        )
                    ysl = slice(half * 8 * C, (half + 1) * 8 * C)
                    nc.scalar.copy(ysb[:, ysl], yps[:])
                if g < NGROUPS - 1:
                    nc.scalar.dma_start(
                        out=y[g],
                        in_=ysb[:].rearrange("p (ch u) -> p ch u", u=C),
                    )
                else:
                    # split the final output DMA so the last HBM write (and
                    # its completion wait) is half-sized
                    half = G * C // 2
                    nc.scalar.dma_start(
                        out=y[g, :, : G // 2],
                        in_=ysb[:, :half].rearrange("p (ch u) -> p ch u", u=C),
                    )
                    nc.scalar.dma_start(
                        out=y[g, :, G // 2 :],
                        in_=ysb[:, half:].rearrange("p (ch u) -> p ch u", u=C),
                    )
    nc.compile()
    return nc


def _get_nc():
    if "nc" not in _NC_CACHE:
        _NC_CACHE["nc"] = _build_nc()
    return _NC_CACHE["nc"]


def kernel(x, alpha, delta, theta, gamma_real, gamma_imag, omega, **_):
    global LAST_EXEC_NS, LAST_RESULTS
    import os

    from concourse.bass_utils import run_bass_kernel_spmd

    x = np.asarray(x)
    sfull = _host_weights(
        np.asarray(alpha), np.asarray(delta), np.asarray(theta),
        np.asarray(gamma_real), np.asarray(gamma_imag), np.asarray(omega),
    )
    # x[b,d,c*C+t'] -> xt[d, t', pad8 + (c,b)] so each channel block's first
    # 8 columns are the zeros the carry matmul view needs.
    xtf = np.zeros((D, C, BLK), dtype=np.float16)
    xtf[:, :, PADB:] = (
        x.reshape(B, D, NCH, C).transpose(1, 3, 2, 0).reshape(D, C, NCH * B)
    )

    nc = _get_nc()
    in_maps = []
    for i in range(NCORES):
        sl = slice(i * DLOC, (i + 1) * DLOC)
        in_maps.append({
            "xt": _group_major(xtf[sl], BLK),
            "seed": _group_major(sfull[sl], SW),
        })
    trace = bool(int(os.environ.get("KERNEL_TRACE", "0")))
    res = run_bass_kernel_spmd(nc, in_maps, list(range(NCORES)), trace=trace)
    LAST_EXEC_NS = res.exec_time_ns
    LAST_RESULTS = res

    y = np.empty((B, D, L), dtype=np.float32)
    for i in range(NCORES):
        yi = res.results[i]["y"]                 # [NGROUPS, (c,b), G, C] fp16
        yi = yi.reshape(NGROUPS, NCH, B, G, C).transpose(2, 0, 3, 1, 4)
        y[:, i * DLOC : (i + 1) * DLOC, :] = (
            yi.reshape(B, DLOC, L).astype(np.float32)
        )
    return y
